# revision 10
# baseline (speedup 1.0000x reference)
"""Trainium2 Bass kernel for nn_FDB_65979287601425 (dual pyramid-pool attention).

Contract: kernel(**inputs) takes FULL inputs (B=16), shards batch across 8
NeuronCores, runs a Bass/Tile kernel per core, returns the FULL output.

Math notes (per batch b; branch A queries x and modulates c, branch C
queries x and modulates a; K/V for branch A from psp(a), for C from psp(c)):
  psp(t)        : concat of g x g adaptive max pools, g in (7,5,3,1) -> [64, 84]
  key_t         = wk_t @ psp(t);  Kc_t = key_t - mean_k(key_t)   (softmax shift)
  M_t           = wq_t^T Kc_t  [64, 84];  brow_t = bq_t^T Kc_t  [1, 84]
  logits_t      = M_ext_t^T @ [x; 1] (ones row folds the q-bias; K=65)
  E_t           = exp(logits_t)                    (ACT, no bias needed)
  s_t           = ones^T E_t   (PE broadcast to 64 rows, both branches
                                stacked into one [128, CH] PSUM tile)
  val2_t        = wfin_t @ (wv_t psp(t) + bv_t)    (final conv folded into V)
  ctx2_t        = val2_t @ E_t                     (stacked [128, CH] PSUM)
  G             = ctx2 * (1/s)      (DVE, one [128, CH] op each)
  T             = G * [c; a]        (DVE, one [128, CH] op)
  out           = x + fold(T) + diag(1+ba) c + diag(1+bc) a   (PE accumulate)
All heavy matmuls run in float32r (full-rate fp32 variant on the PE).
Pooling max cascade runs on GPSIMD to keep the DVE free for the main loop.
"""

import sys

sys.path.insert(0, "/opt/trn_rl_repo")

import numpy as np

import concourse.bass as bass
import concourse.bacc as bacc
import concourse.tile as tile
from concourse import mybir
from concourse.bass_utils import run_bass_kernel_spmd

f32 = mybir.dt.float32
f32r = mybir.dt.float32r
bf16 = mybir.dt.bfloat16
FT = mybir.ActivationFunctionType
OP = mybir.AluOpType
AX = mybir.AxisListType

B, C, H, W = 16, 64, 210, 210
HW = H * W                 # 44100
KS = 84                    # pyramid key slots (49+25+9+1)
NCORES = 8
NB = B // NCORES           # batches per core
CH = 490                   # main-loop chunk columns (44100 % 490 == 0)
PW = 512                   # psum-bank-aligned stride for paired regions
NPAIR = HW // (2 * CH)     # 45 chunk-pairs
RP = 14                    # pooling rows per chunk (even; 210 % 14 == 0)
NPCH = H // RP             # 15 pooling chunks
H2R = RP // 2              # 7 h2-rows per pooling chunk


def _build_nc(reps=1):
    nc = bacc.Bacc(trn_type="TRN2")

    xe_d = nc.declare_dram_parameter("xe", [NB, 65, H, W], f32, isOutput=False)
    ms_d = nc.declare_dram_parameter("ms", [NB, 128, H, W], f32, isOutput=False)
    wq_d = nc.declare_dram_parameter("wq", [128, 64], f32, isOutput=False)
    wkT_d = nc.declare_dram_parameter("wkT", [128, 64], f32, isOutput=False)
    wvT_d = nc.declare_dram_parameter("wvT", [128, 64], f32, isOutput=False)
    wfT_d = nc.declare_dram_parameter("wfT", [128, 64], f32, isOutput=False)
    ident2_d = nc.declare_dram_parameter("ident2", [128, 64], f32, isOutput=False)
    diagb_d = nc.declare_dram_parameter("diagb", [128, 64], f32, isOutput=False)
    ones_d = nc.declare_dram_parameter("ones84", [KS, 64], f32, isOutput=False)
    bq_d = nc.declare_dram_parameter("bq", [128, 1], f32, isOutput=False)
    bv_d = nc.declare_dram_parameter("bv", [128, 1], f32, isOutput=False)
    out_d = nc.declare_dram_parameter("out", [NB, C, H, W], f32, isOutput=True)

    from contextlib import ExitStack

    with tile.TileContext(nc) as tc:
        with ExitStack() as ctx:
            ep = lambda **kw: ctx.enter_context(tc.tile_pool(**kw))
            cst = ep(name="cst", bufs=1)
            # pooling pools
            praw = ep(name="praw", bufs=2)
            ph2 = ep(name="ph2", bufs=2)
            pw2 = ep(name="pw2", bufs=2)
            pw6 = ep(name="pw6", bufs=2)
            pwfull = ep(name="pwfull", bufs=2)
            ppool = ep(name="ppool", bufs=2)
            # preamble pools
            ppre = ep(name="ppre", bufs=1, space="PSUM")
            spre = ep(name="spre", bufs=4)
            slhs = ep(name="slhs", bufs=4)
            # main-loop pools
            mx = ep(name="mx", bufs=3)
            mmd = ep(name="mmd", bufs=3)
            me = ep(name="me", bufs=2)
            mb = ep(name="mb", bufs=2)
            mg = ep(name="mg", bufs=2)
            mt = ep(name="mt", bufs=2)
            mo = ep(name="mo", bufs=2)
            pl = ep(name="pl", bufs=1, space="PSUM")
            psS = ep(name="psS", bufs=1, space="PSUM")
            psC = ep(name="psC", bufs=1, space="PSUM")
            po = ep(name="po", bufs=1, space="PSUM")
            # ---- constants ----
            wq_t = cst.tile([128, 64], f32)
            nc.sync.dma_start(out=wq_t[:, :], in_=wq_d[:, :])
            wkT_t = cst.tile([128, 64], f32)
            nc.sync.dma_start(out=wkT_t[:, :], in_=wkT_d[:, :])
            wvT_t = cst.tile([128, 64], f32)
            nc.sync.dma_start(out=wvT_t[:, :], in_=wvT_d[:, :])
            wfT_t = cst.tile([128, 64], f32)
            nc.sync.dma_start(out=wfT_t[:, :], in_=wfT_d[:, :])
            bq_t = cst.tile([128, 1], f32)
            nc.sync.dma_start(out=bq_t[:, :], in_=bq_d[:, :])
            bv_t = cst.tile([128, 1], f32)
            nc.sync.dma_start(out=bv_t[:, :], in_=bv_d[:, :])
            # f32r-consumed constants
            ones84_t = cst.tile([KS, 64], f32)
            nc.sync.dma_start(out=ones84_t[:, :], in_=ones_d[:, :])
            ones84_b = cst.tile([KS, 64], bf16)
            nc.vector.tensor_copy(ones84_b[:, :], ones84_t[:, :])
            identR2_t = cst.tile([128, 64], f32)
            nc.sync.dma_start(out=identR2_t[:, :].bitcast(f32r), in_=ident2_d[:, :].bitcast(f32r))
            diagb_t = cst.tile([128, 64], f32)
            nc.sync.dma_start(out=diagb_t[:, :].bitcast(f32r), in_=diagb_d[:, :].bitcast(f32r))

            def pooling(b):
                """psp over ms[b] -> pooled [128, 84] (rows 0:64 = psp(c), 64:128 = psp(a))."""
                w30 = pwfull.tile([128, 105, 7], f32, tag="w30")
                w42 = pwfull.tile([128, 105, 5], f32, tag="w42")
                w70 = pwfull.tile([128, 105, 3], f32, tag="w70")
                for ci in range(NPCH):
                    r0 = ci * RP
                    raw = praw.tile([128, RP, W], f32)
                    nc.sync.dma_start(out=raw[:, :, :], in_=ms_d[b, :, r0 : r0 + RP, :])
                    rawv = raw.rearrange("p (h two) w -> p h two w", two=2)
                    h2 = ph2.tile([128, H2R, W], f32)
                    nc.vector.tensor_tensor(h2[:, :, :], rawv[:, :, 0, :], rawv[:, :, 1, :], OP.max)
                    h2v = h2.rearrange("p h (w two) -> p h w two", two=2)
                    w2 = pw2.tile([128, H2R, 105], f32)
                    nc.vector.tensor_tensor(w2[:, :, :], h2v[:, :, :, 0], h2v[:, :, :, 1], OP.max)
                    w6 = pw6.tile([128, H2R, 35], f32)
                    nc.vector.reduce_max(
                        w6[:, :, :], w2.rearrange("p h (w win) -> p h w win", win=3), axis=AX.X
                    )
                    hsl = slice(ci * H2R, (ci + 1) * H2R)
                    nc.vector.reduce_max(
                        w30[:, hsl, :], w6.rearrange("p h (w win) -> p h w win", win=5), axis=AX.X
                    )
                    nc.vector.reduce_max(
                        w42[:, hsl, :], w6.rearrange("p h (w win) -> p h w win", win=7), axis=AX.X
                    )
                    nc.vector.reduce_max(
                        w70[:, hsl, :], w2.rearrange("p h (w win) -> p h w win", win=35), axis=AX.X
                    )
                pooled = ppool.tile([128, KS], f32)
                nc.vector.reduce_max(
                    pooled[:, 0:49], w30.rearrange("p (hb h2) w -> p hb w h2", hb=7), axis=AX.X
                )
                nc.vector.reduce_max(
                    pooled[:, 49:74], w42.rearrange("p (hb h2) w -> p hb w h2", hb=5), axis=AX.X
                )
                nc.vector.reduce_max(
                    pooled[:, 74:83], w70.rearrange("p (hb h2) w -> p hb w h2", hb=3), axis=AX.X
                )
                nc.vector.reduce_max(
                    pooled[:, 83:84], w70.rearrange("p a b -> p (a b)"), axis=AX.X
                )
                return pooled

            def preamble(pooled):
                """Per-batch K/V prep. Returns (MaT_ext[A,C], val2T[A,C]).

                pr rows 0:64 = C-branch (keys/vals from psp(c)), 64:128 = A-branch.
                """
                kps = ppre.tile([128, KS], f32, tag="pre")
                nc.tensor.matmul(kps[0:64, :], wkT_t[0:64, :], pooled[0:64, :])
                nc.tensor.matmul(kps[64:128, :], wkT_t[64:128, :], pooled[64:128, :], tile_position=(64, 64))
                nmean = spre.tile([128, 1], f32, tag="nmean")
                nc.vector.tensor_reduce(nmean[:, :], kps[:, :], axis=AX.X, op=OP.add, negate=True)
                nc.scalar.mul(nmean[:, :], nmean[:, :], 1.0 / KS)
                kt = spre.tile([128, KS], f32, tag="kt")
                nc.vector.tensor_scalar_add(kt[:, :], kps[:, :], nmean[:, :])
                vps = ppre.tile([128, KS], f32, tag="pre")
                nc.tensor.matmul(vps[0:64, :], wvT_t[0:64, :], pooled[0:64, :])
                nc.tensor.matmul(vps[64:128, :], wvT_t[64:128, :], pooled[64:128, :], tile_position=(64, 64))
                va = spre.tile([128, KS], f32, tag="va")
                nc.scalar.activation(va[:, :], vps[:, :], FT.Identity, bias=bv_t[:, :])

                MaTs, vTs = {}, {}
                for br, name in ((0, "C"), (1, "A")):
                    pr = slice(64 * br, 64 * br + 64)
                    mps = ppre.tile([64, KS], f32, tag="pre")
                    nc.tensor.matmul(mps[:, :], wq_t[pr, :], kt[pr, :], tile_position=(64 * br, 0))
                    MaT = slhs.tile([65, KS], f32, tag="mat" + name)
                    nc.scalar.copy(MaT[0:64, :].bitcast(f32r), mps[:, :])
                    brow = ppre.tile([1, KS], f32, tag="pre")
                    nc.tensor.matmul(brow[:, :], bq_t[pr, :], kt[pr, :], tile_position=(64 * br, 0))
                    nc.scalar.copy(MaT[64:65, :].bitcast(f32r), brow[:, :])
                    vps2 = ppre.tile([KS, 64], f32, tag="pre")
                    nc.tensor.matmul(vps2[:, :], va[pr, :], wfT_t[pr, :], tile_position=(64 * br, 0))
                    vT = slhs.tile([KS, 64], bf16, tag="vt" + name)
                    nc.scalar.copy(vT[:, :], vps2[:, :])
                    MaTs[name] = MaT
                    vTs[name] = vT
                return MaTs, vTs

            def main_loop(b, MaTs, vTs):
                xf = xe_d[b].rearrange("c h w -> c (h w)")
                mf = ms_d[b].rearrange("c h w -> c (h w)")
                of = out_d[b].rearrange("c h w -> c (h w)")
                for p in range(NPAIR):
                    sl2 = slice(2 * CH * p, 2 * CH * (p + 1))
                    x2 = mx.tile([65, 2 * CH], f32)
                    nc.sync.dma_start(out=x2[:, :].bitcast(f32r), in_=xf[:, sl2].bitcast(f32r))
                    md = mmd.tile([128, 2 * CH], f32)
                    nc.sync.dma_start(out=md[:, :].bitcast(f32r), in_=mf[:, sl2].bitcast(f32r))
                    ot = mo.tile([64, 2 * CH], f32)
                    # pair-packed PSUM accumulators: cols [0:CH] = sub0, [PW:PW+CH] = sub1,
                    # partitions 0:64 = branch A, 64:128 = branch C
                    Ts = psS.tile([128, 2 * PW], f32, tag="ts")
                    Tc = psC.tile([128, 2 * PW], f32, tag="tc")
                    E2 = []
                    for sub in range(2):
                        csl = slice(CH * sub, CH * (sub + 1))
                        psl = slice(PW * sub, PW * sub + CH)
                        Lg = pl.tile([KS, 2 * PW], f32, tag="lg")
                        nc.tensor.matmul(
                            Lg[:, 0:CH], MaTs["A"][:, :].bitcast(f32r), x2[:, csl].bitcast(f32r)
                        )
                        nc.tensor.matmul(
                            Lg[:, PW : PW + CH], MaTs["C"][:, :].bitcast(f32r), x2[:, csl].bitcast(f32r)
                        )
                        E = me.tile([KS, 2 * PW], bf16)
                        nc.scalar.activation(
                            E[:, 0 : PW + CH], Lg[:, 0 : PW + CH], FT.Exp
                        )
                        E2.append(E)
                        # branch A (keys at E[:, 0:CH]) -> partitions 0:64, one K=84 matmul
                        nc.tensor.matmul(Ts[0:64, psl], ones84_b[:, :], E[:, 0:CH])
                        nc.tensor.matmul(Tc[0:64, psl], vTs["A"][:, :], E[:, 0:CH])
                        # branch C -> partitions 64:128: K split 64+20 (128-row tiles
                        # cannot target upper PSUM partitions; 64-row quadrants can)
                        ec = E[:, PW : PW + CH]
                        nc.tensor.matmul(
                            Ts[64:128, psl], ones84_b[0:64, :], ec[0:64, :],
                            start=True, stop=False, tile_position=(0, 64),
                        )
                        nc.tensor.matmul(
                            Ts[64:128, psl], ones84_b[64:KS, :], ec[64:KS, :],
                            start=False, stop=True, tile_position=(64, 64),
                        )
                        nc.tensor.matmul(
                            Tc[64:128, psl], vTs["C"][0:64, :], ec[0:64, :],
                            start=True, stop=False, tile_position=(0, 64),
                        )
                        nc.tensor.matmul(
                            Tc[64:128, psl], vTs["C"][64:KS, :], ec[64:KS, :],
                            start=False, stop=True, tile_position=(64, 64),
                        )
                    # per-pair elementwise on [128, ~1002] (garbage cols 490:512 unused)
                    binv = mb.tile([128, 2 * PW], f32)
                    nc.vector.reciprocal_approx_fast(binv[:, 0 : PW + CH], Ts[:, 0 : PW + CH])
                    G = mg.tile([128, 2 * PW], f32)
                    nc.vector.tensor_tensor(
                        G[:, 0 : PW + CH], Tc[:, 0 : PW + CH], binv[:, 0 : PW + CH], OP.mult
                    )
                    T = mt.tile([128, 2 * PW], f32)
                    Tv = T.rearrange("p (two w) -> p two w", two=2)
                    Gv = G.rearrange("p (two w) -> p two w", two=2)
                    mdv = md.rearrange("p (two w) -> p two w", two=2)
                    nc.vector.tensor_tensor(
                        Tv[:, :, 0:CH].bitcast(f32r), Gv[:, :, 0:CH], mdv[:, :, :], OP.mult
                    )
                    for sub in range(2):
                        csl = slice(CH * sub, CH * (sub + 1))
                        psl = slice(PW * sub, PW * sub + CH)
                        pO = po.tile([64, PW], f32, tag="po")
                        nc.tensor.matmul(
                            pO[:, 0:CH], identR2_t[:, :].bitcast(f32r), T[:, psl].bitcast(f32r),
                            start=True, stop=False,
                        )
                        nc.tensor.matmul(
                            pO[:, 0:CH], diagb_t[:, :].bitcast(f32r), md[:, csl].bitcast(f32r),
                            start=False, stop=False,
                        )
                        nc.tensor.matmul(
                            pO[:, 0:CH], identR2_t[0:64, :].bitcast(f32r),
                            x2[0:64, csl].bitcast(f32r),
                            start=False, stop=True,
                        )
                        nc.scalar.copy(ot[:, csl], pO[:, 0:CH])
                    nc.sync.dma_start(out=of[:, sl2], in_=ot[:, :])

            for _rep in range(reps):
                pooled0 = pooling(0)
                pre0 = preamble(pooled0)
                pooled1 = pooling(1)
                main_loop(0, *pre0)
                pre1 = preamble(pooled1)
                main_loop(1, *pre1)

    nc.compile()
    return nc


_NC_CACHE_R = {}


def _get_nc_reps(reps):
    if reps not in _NC_CACHE_R:
        _NC_CACHE_R[reps] = _build_nc(reps)
    return _NC_CACHE_R[reps]


def _get_nc():
    return _get_nc_reps(1)


def _make_consts(inputs):
    eye = np.eye(64, dtype=np.float32)
    consts = {
        # branch order: rows 0:64 = C-branch, rows 64:128 = A-branch
        "wq": np.concatenate([inputs["wqc"], inputs["wqa"]], axis=0),
        "wkT": np.concatenate([inputs["wkc"].T, inputs["wka"].T], axis=0),
        "wvT": np.concatenate([inputs["wvc"].T, inputs["wva"].T], axis=0),
        "wfT": np.concatenate([inputs["wc"].T, inputs["wa"].T], axis=0),
        "ident2": np.concatenate([eye, eye], axis=0),
        # combine: md rows 0:64 = c gets diag(1+ba); rows 64:128 = a gets diag(1+bc)
        "diagb": np.concatenate(
            [np.diag(1.0 + inputs["ba"]), np.diag(1.0 + inputs["bc"])], axis=0
        ),
        "ones84": np.ones((KS, 64), dtype=np.float32),
        "bq": np.concatenate([inputs["bqc"], inputs["bqa"]])[:, None],
        "bv": np.concatenate([inputs["bvc"], inputs["bva"]])[:, None],
    }
    return {k: np.ascontiguousarray(v, dtype=np.float32) for k, v in consts.items()}


def make_in_maps(inputs):
    inputs = {k: np.ascontiguousarray(np.asarray(v), dtype=np.float32) for k, v in inputs.items()}
    x, a, c = inputs["x"], inputs["a"], inputs["c"]
    ones_row = np.ones((B, 1, H, W), dtype=np.float32)
    xe = np.concatenate([x, ones_row], axis=1)        # [B, 65, H, W]
    ms = np.concatenate([c, a], axis=1)               # [B, 128, H, W]
    consts = _make_consts(inputs)
    in_maps = []
    for j in range(NCORES):
        sl = slice(NB * j, NB * (j + 1))
        m = {"xe": xe[sl], "ms": ms[sl]}
        m.update(consts)
        in_maps.append(m)
    return in_maps


def kernel(**inputs):
    in_maps = make_in_maps(inputs)
    nc = _get_nc()
    res = run_bass_kernel_spmd(nc, in_maps, list(range(NCORES)))
    out = np.concatenate([res.results[j]["out"] for j in range(NCORES)], axis=0)
    return out
